# revision 1
# baseline (speedup 1.0000x reference)
"""Trainium2 Bass kernel for nn_BilinearFeedForward.

reference (B=4, N=2048, D=1024, fp32):
    query = (x_real @ Wqr) * (x_imag @ Wqi)            # [B,N,D]
    key   = x_real @ Wk ; value = x_imag @ Wv          # [B,N,D]
    key   /= max(||key||_n, eps) ; value /= max(||value||_n, eps)   (norm over N)
    kv    = einsum('bnd,bne->bde', key, value)         # [B,D,D]
    out   = einsum('bnd,bde->bne', query, kv) + bias   # [B,N,D]

Sharding: 8 cores = (batch b in 0..3) x (e-half eh in 0..1).  Each core
computes out[b, :, eh*512:(eh+1)*512] with zero collectives:
  - full-width K, Qr, Qi for its batch, half-width V
  - normalization folded into scalars:  kv = diag(1/sk) (K^T V) diag(1/sv)
  - everything runs transposed so per-feature scalars live on partitions.

Host pre-transposes x to x.T per batch (feeds the PE contraction layout)
and re-transposes the per-core [512, 2048] outputs.
"""

import os
import sys
import numpy as np

for _p in ("/opt/trn_rl_repo", "/root/.axon_site/_ro/trn_rl_repo"):
    if _p not in sys.path and os.path.isdir(_p):
        sys.path.append(_p)

# Some images lack antenv.axon_hooks; bass_utils imports it unconditionally
# when BASS_TRACE is set.  Provide a degrade-to-no-trace shim if missing.
try:
    import antenv.axon_hooks  # noqa: F401
except Exception:
    import types

    try:
        import antenv

        _hooks = types.ModuleType("antenv.axon_hooks")
        _hooks._hook = None
        _hooks.get_axon_ntff_profile_hook = lambda: _hooks._hook

        def _set_hook(h):
            _hooks._hook = h

        _hooks.set_axon_ntff_profile_hook = _set_hook
        sys.modules["antenv.axon_hooks"] = _hooks
        antenv.axon_hooks = _hooks
    except Exception:
        pass

B, N, D = 4, 2048, 1024
EH = 512          # e-half width
P = 128
DT = D // P       # 8 d-tiles
ET = EH // P      # 4 e-tiles of the half
NCK = 512         # n-chunk
CHUNKS = N // NCK # 4
EPS = 1e-5

_CACHE = {}
LAST_EXEC_NS = None


def _build_bass():
    import concourse.bacc as bacc
    import concourse.tile as tile
    import concourse.mybir as mybir

    f32 = mybir.dt.float32
    f32r = mybir.dt.float32r
    bf16 = mybir.dt.bfloat16
    Act = mybir.ActivationFunctionType
    Alu = mybir.AluOpType

    nc = bacc.Bacc()

    xrt_d = nc.dram_tensor("xrt", [D, N], f32, kind="ExternalInput")
    xit_d = nc.dram_tensor("xit", [D, N], f32, kind="ExternalInput")
    wqr_d = nc.dram_tensor("wqr", [D, D], f32, kind="ExternalInput")
    wqi_d = nc.dram_tensor("wqi", [D, D], f32, kind="ExternalInput")
    wk_d = nc.dram_tensor("wk", [D, D], f32, kind="ExternalInput")
    wv_d = nc.dram_tensor("wv", [D, EH], f32, kind="ExternalInput")
    bias_d = nc.dram_tensor("bias", [EH], f32, kind="ExternalInput")
    out_d = nc.dram_tensor("out_t", [EH, N], f32, kind="ExternalOutput")

    xrt_r = xrt_d.rearrange("(t p) n -> p t n", p=P)
    xit_r = xit_d.rearrange("(t p) n -> p t n", p=P)
    wqr_r = wqr_d.rearrange("(t p) e -> p t e", p=P)
    wqi_r = wqi_d.rearrange("(t p) e -> p t e", p=P)
    wk_r = wk_d.rearrange("(t p) e -> p t e", p=P)
    wv_r = wv_d.rearrange("(t p) e -> p t e", p=P)
    bias_r = bias_d.rearrange("(t p) -> p t", p=P)
    out_r = out_d.rearrange("(t p) n -> p t n", p=P)

    with tile.TileContext(nc) as tc:
        with tc.tile_pool(name="outer", bufs=1) as outer:
            a_sb = outer.tile([P, DT, EH], f32, tag="a_sb")
            skinv = outer.tile([P, DT], f32, tag="skinv")
            svinv = outer.tile([P, ET], f32, tag="svinv")
            bias_pp = outer.tile([P, ET], f32, tag="bias_pp")
            zero32 = outer.tile([P, 1], f32, tag="zero32")
            ones_bf = outer.tile([P, 1], bf16, tag="ones_bf")
            nc.vector.memset(zero32[:], 0.0)
            nc.vector.memset(ones_bf[:], 1.0)
            nc.sync.dma_start(out=bias_pp[:], in_=bias_r)

            sk2_sb = outer.tile([P, DT], f32, tag="sk2_sb")
            sv2_sb = outer.tile([P, ET], f32, tag="sv2_sb")
            nc.vector.memset(sk2_sb[:], 0.0)
            nc.vector.memset(sv2_sb[:], 0.0)

            # ---------------- Phase A: K, V, A = K^T V, norms ----------------
            NCKA = 256
            CHA = N // NCKA
            NTA = NCKA // P
            ctx_wq = tc.tile_pool(name="wq", bufs=1)
            wq = ctx_wq.__enter__()
            wqr_sb = wq.tile([P, DT, D], f32r, tag="wqr")
            wqi_sb = wq.tile([P, DT, D], f32r, tag="wqi")
            with (
                tc.tile_pool(name="wkv", bufs=1) as wkv,
                tc.tile_pool(name="xin", bufs=2) as xin,
                tc.tile_pool(name="kv", bufs=2) as kvp,
                tc.tile_pool(name="sqp", bufs=4) as sqp,
                tc.tile_pool(name="ps_kv", bufs=3, space="PSUM") as ps_kv,
                tc.tile_pool(name="ps_a", bufs=2, space="PSUM") as ps_a,
                tc.tile_pool(name="nrm_ps", bufs=3, space="PSUM") as nrm_ps,
            ):
                wk_sb = wkv.tile([P, DT, D], f32r, tag="wk")
                wv_sb = wkv.tile([P, DT, EH], f32r, tag="wv")

                for c4 in range(CHA):
                    ns = c4 * NCKA
                    xrt_c = xin.tile([P, DT, NCKA], f32r, tag="xrt_c")
                    xit_c = xin.tile([P, DT, NCKA], f32r, tag="xit_c")
                    if c4 == 0:
                        # startup: split the K-path inputs (xrt + wk) across
                        # both DMA queues so the first chains unblock fastest
                        nc.sync.dma_start(out=xrt_c[:, 0:4, :],
                                          in_=xrt_r[:, 0:4, ns:ns + NCKA].bitcast(f32r))
                        nc.gpsimd.dma_start(out=xrt_c[:, 4:8, :],
                                            in_=xrt_r[:, 4:8, ns:ns + NCKA].bitcast(f32r))
                        for t in range(DT):
                            eng = nc.sync if t % 2 else nc.gpsimd
                            eng.dma_start(out=wk_sb[:, t, 0:512],
                                          in_=wk_r[:, t, 0:512].bitcast(f32r))
                        for t in range(DT):
                            eng = nc.sync if t % 2 else nc.gpsimd
                            eng.dma_start(out=wk_sb[:, t, 512:1024],
                                          in_=wk_r[:, t, 512:1024].bitcast(f32r))
                        nc.gpsimd.dma_start(out=xit_c[:],
                                            in_=xit_r[:, :, ns:ns + NCKA].bitcast(f32r))
                        for t in range(DT):
                            eng = nc.gpsimd if t % 2 else nc.sync
                            eng.dma_start(out=wv_sb[:, t, :],
                                          in_=wv_r[:, t, :].bitcast(f32r))
                    else:
                        nc.sync.dma_start(out=xrt_c[:], in_=xrt_r[:, :, ns:ns + NCKA].bitcast(f32r))
                        nc.gpsimd.dma_start(out=xit_c[:], in_=xit_r[:, :, ns:ns + NCKA].bitcast(f32r))
                    if c4 == 3:
                        # prefetch phase-C weights in the DMA slack mid-phase
                        for t in range(DT):
                            nc.sync.dma_start(out=wqr_sb[:, t, :],
                                              in_=wqr_r[:, t, :].bitcast(f32r))
                            nc.gpsimd.dma_start(out=wqi_sb[:, t, :],
                                                in_=wqi_r[:, t, :].bitcast(f32r))

                    k_c = kvp.tile([P, NTA, D], f32r, tag="k_c")
                    v_c = kvp.tile([P, NTA, EH], f32r, tag="v_c")

                    first = c4 == 0
                    for nt in range(NTA):
                        nsl = slice(nt * P, (nt + 1) * P)
                        sqs = []
                        for do2 in range(2):
                            kps = ps_kv.tile([P, 512], f32, tag="kvps")
                            for t in range(DT):
                                nc.tensor.matmul(
                                    kps[:], xrt_c[:, t, nsl],
                                    wk_sb[:, t, do2 * 512:(do2 + 1) * 512],
                                    start=(t == 0), stop=(t == DT - 1))
                            nc.vector.tensor_copy(
                                out=k_c[:, nt, do2 * 512:(do2 + 1) * 512], in_=kps[:])
                            sq = sqp.tile([P, 512], bf16, tag="sq")
                            nc.scalar.activation(out=sq[:], in_=kps[:], func=Act.Square,
                                                 bias=zero32[:], scale=1.0)
                            sqs.append(sq)
                        vps = ps_kv.tile([P, 512], f32, tag="kvps")
                        for t in range(DT):
                            nc.tensor.matmul(vps[:], xit_c[:, t, nsl], wv_sb[:, t, :],
                                             start=(t == 0), stop=(t == DT - 1))
                        nc.vector.tensor_copy(out=v_c[:, nt, :], in_=vps[:])
                        sqv = sqp.tile([P, 512], bf16, tag="sq")
                        nc.scalar.activation(out=sqv[:], in_=vps[:], func=Act.Square,
                                             bias=zero32[:], scale=1.0)
                        # batched norm matmuls: one pipeline break per n-tile
                        for do2 in range(2):
                            nps = nrm_ps.tile([P, 4], f32, tag="nps")
                            for j in range(4):
                                nc.tensor.matmul(
                                    nps[:, j:j + 1], sqs[do2][:, j * P:(j + 1) * P],
                                    ones_bf[:], start=True, stop=True)
                            nc.vector.tensor_add(
                                out=sk2_sb[:, do2 * 4:(do2 + 1) * 4],
                                in0=sk2_sb[:, do2 * 4:(do2 + 1) * 4], in1=nps[:])
                        npsv = nrm_ps.tile([P, 4], f32, tag="nps")
                        for j in range(4):
                            nc.tensor.matmul(
                                npsv[:, j:j + 1], sqv[:, j * P:(j + 1) * P],
                                ones_bf[:], start=True, stop=True)
                        nc.vector.tensor_add(out=sv2_sb[:], in0=sv2_sb[:], in1=npsv[:])

                    # A += K_c^T @ V_c  (contract the chunk's rows)
                    for dt in range(DT):
                        aps = ps_a.tile([P, EH], f32, tag="aps")
                        for nt in range(NTA):
                            nc.tensor.matmul(aps[:], k_c[:, nt, dt * P:(dt + 1) * P],
                                             v_c[:, nt, :], start=(nt == 0),
                                             stop=(nt == NTA - 1))
                        if first:
                            nc.vector.tensor_copy(out=a_sb[:, dt, :].bitcast(f32r),
                                                  in_=aps[:])
                        else:
                            nc.vector.tensor_add(out=a_sb[:, dt, :].bitcast(f32r),
                                                 in0=a_sb[:, dt, :], in1=aps[:])

            # ---------------- Phase B: finalize norms, scale A ----------------
            nc.scalar.activation(out=skinv[:], in_=sk2_sb[:], func=Act.Sqrt,
                                 bias=zero32[:], scale=1.0)
            nc.vector.tensor_scalar_max(skinv[:], skinv[:], EPS)
            nc.vector.reciprocal(skinv[:], skinv[:])
            nc.scalar.activation(out=svinv[:], in_=sv2_sb[:], func=Act.Sqrt,
                                 bias=zero32[:], scale=1.0)
            nc.vector.tensor_scalar_max(svinv[:], svinv[:], EPS)
            nc.vector.reciprocal(svinv[:], svinv[:])
            for dt in range(DT):
                nc.vector.tensor_scalar_mul(
                    out=a_sb[:, dt, :].bitcast(f32r), in0=a_sb[:, dt, :],
                    scalar1=skinv[:, dt:dt + 1])

            # ---------------- Phase C: Q^T and out^T = (A')^T Q^T ------------
            with (
                tc.tile_pool(name="xin2", bufs=2) as xin2,
                tc.tile_pool(name="qrp", bufs=3) as qrp,
                tc.tile_pool(name="qtp", bufs=2) as qtp,
                tc.tile_pool(name="outp", bufs=3) as outp,
                tc.tile_pool(name="ps_q", bufs=4, space="PSUM") as ps_q,
                tc.tile_pool(name="ps_o", bufs=2, space="PSUM") as ps_o,
            ):
                for c4 in range(CHUNKS):
                    ns = c4 * NCK
                    xrt_c = xin2.tile([P, DT, NCK], f32r, tag="xrt_c2")
                    xit_c = xin2.tile([P, DT, NCK], f32r, tag="xit_c2")
                    nc.sync.dma_start(out=xrt_c[:], in_=xrt_r[:, :, ns:ns + NCK].bitcast(f32r))
                    nc.gpsimd.dma_start(out=xit_c[:], in_=xit_r[:, :, ns:ns + NCK].bitcast(f32r))

                    qt_c = qtp.tile([P, DT, NCK], f32r, tag="qt_c")
                    for dqt in range(DT):
                        qsl = slice(dqt * P, (dqt + 1) * P)
                        qrps = ps_q.tile([P, NCK], f32, tag="qps")
                        for t in range(DT):
                            nc.tensor.matmul(qrps[:], wqr_sb[:, t, qsl], xrt_c[:, t, :],
                                             start=(t == 0), stop=(t == DT - 1))
                        qr_sb = qrp.tile([P, NCK], f32, tag="qr_sb")
                        nc.scalar.activation(out=qr_sb[:], in_=qrps[:], func=Act.Copy,
                                             bias=0.0, scale=1.0)
                        qips = ps_q.tile([P, NCK], f32, tag="qps")
                        for t in range(DT):
                            nc.tensor.matmul(qips[:], wqi_sb[:, t, qsl], xit_c[:, t, :],
                                             start=(t == 0), stop=(t == DT - 1))
                        nc.vector.tensor_mul(out=qt_c[:, dqt, :], in0=qips[:], in1=qr_sb[:])

                    for et in range(ET):
                        esl = slice(et * P, (et + 1) * P)
                        ops_t = ps_o.tile([P, NCK], f32, tag="ops")
                        for dt in range(DT):
                            nc.tensor.matmul(ops_t[:], a_sb[:, dt, esl].bitcast(f32r),
                                             qt_c[:, dt, :],
                                             start=(dt == 0), stop=(dt == DT - 1))
                        out_sb = outp.tile([P, NCK], f32, tag="out_sb")
                        nc.vector.tensor_scalar(
                            out=out_sb[:], in0=ops_t[:],
                            scalar1=svinv[:, et:et + 1], scalar2=bias_pp[:, et:et + 1],
                            op0=Alu.mult, op1=Alu.add)
                        nc.sync.dma_start(out=out_r[:, et, ns:ns + NCK], in_=out_sb[:])

            ctx_wq.__exit__(None, None, None)

    nc.finalize()
    return nc


def kernel(x_real, x_imag, w_query_real, w_query_imag, w_key, w_value, bias):
    global LAST_EXEC_NS
    from concourse.bass_utils import run_bass_kernel_spmd

    x_real = np.ascontiguousarray(np.asarray(x_real, dtype=np.float32))
    x_imag = np.ascontiguousarray(np.asarray(x_imag, dtype=np.float32))
    wqr = np.ascontiguousarray(np.asarray(w_query_real, dtype=np.float32))
    wqi = np.ascontiguousarray(np.asarray(w_query_imag, dtype=np.float32))
    wk = np.ascontiguousarray(np.asarray(w_key, dtype=np.float32))
    wv = np.ascontiguousarray(np.asarray(w_value, dtype=np.float32))
    bias = np.ascontiguousarray(np.asarray(bias, dtype=np.float32))

    nc = _CACHE.get("nc")
    if nc is None:
        nc = _build_bass()
        _CACHE["nc"] = nc

    xrt = [np.ascontiguousarray(x_real[b].T) for b in range(B)]
    xit = [np.ascontiguousarray(x_imag[b].T) for b in range(B)]
    wv_h = [np.ascontiguousarray(wv[:, eh * EH:(eh + 1) * EH]) for eh in range(2)]
    bias_h = [np.ascontiguousarray(bias[eh * EH:(eh + 1) * EH]) for eh in range(2)]

    in_maps = []
    for c in range(8):
        b, eh = c // 2, c % 2
        in_maps.append({
            "xrt": xrt[b], "xit": xit[b],
            "wqr": wqr, "wqi": wqi, "wk": wk,
            "wv": wv_h[eh], "bias": bias_h[eh],
        })

    res = run_bass_kernel_spmd(nc, in_maps, list(range(8)))
    LAST_EXEC_NS = res.exec_time_ns

    out = np.empty((B, N, D), dtype=np.float32)
    for c in range(8):
        b, eh = c // 2, c % 2
        out[b, :, eh * EH:(eh + 1) * EH] = np.asarray(res.results[c]["out_t"]).T
    return out



# revision 2
# speedup vs baseline: 1.0386x; 1.0386x over previous
"""Trainium2 Bass kernel for nn_BilinearFeedForward — n-split + pipelined CC.

Sharding: 8 cores = (batch b) x (n-half h).  Each core does the ideal
6.45G MACs: K, V, Q, out for its own 1024 rows; partial kv^T and column
norms pairwise AllReduce'd in bf16.

Phase A is restructured V-first, then K and the kv rows by d-superblock
(4 blocks x 2 d-tiles), so each block's partial-kv rows + norms are
final early and its 525KB AllReduce piece is issued mid-phase-A.  The
4-piece CC pipeline overlaps phase A's tail and the whole Q phase.

All matmuls bf16 (same PE rate as f32r, half DMA/SBUF), PSUM fp32.
"""

import os
import sys
import numpy as np

for _p in ("/opt/trn_rl_repo", "/root/.axon_site/_ro/trn_rl_repo"):
    if _p not in sys.path and os.path.isdir(_p):
        sys.path.append(_p)

try:
    import antenv.axon_hooks  # noqa: F401
except Exception:
    import types

    try:
        import antenv

        _hooks = types.ModuleType("antenv.axon_hooks")
        _hooks._hook = None
        _hooks.get_axon_ntff_profile_hook = lambda: _hooks._hook

        def _set_hook(h):
            _hooks._hook = h

        _hooks.set_axon_ntff_profile_hook = _set_hook
        sys.modules["antenv.axon_hooks"] = _hooks
        antenv.axon_hooks = _hooks
    except Exception:
        pass

B, N, D = 4, 2048, 1024
NH = N // 2       # rows per core
P = 128
DT = D // P       # 8 d-tiles
NTT = NH // P     # 8 n-tiles total
NCK = 512         # n-chunk for V / Q / out phases
CHUNKS = NH // NCK  # 2
NBLK = 4          # kv d-superblocks
BDT = DT // NBLK  # 2 d-tiles per block
AW = D + 2        # CC row width: 1024 kv cols + sk2 + sv2
EPS = 1e-5

_CACHE = {}
LAST_EXEC_NS = None


def _build_bass():
    import concourse.bacc as bacc
    import concourse.tile as tile
    import concourse.mybir as mybir

    f32 = mybir.dt.float32
    bf16 = mybir.dt.bfloat16
    Act = mybir.ActivationFunctionType
    Alu = mybir.AluOpType

    nc = bacc.Bacc(num_devices=8)

    xrt_d = nc.dram_tensor("xrt", [D, NH], bf16, kind="ExternalInput")
    xit_d = nc.dram_tensor("xit", [D, NH], bf16, kind="ExternalInput")
    wqr_d = nc.dram_tensor("wqr", [D, D], bf16, kind="ExternalInput")
    wqi_d = nc.dram_tensor("wqi", [D, D], bf16, kind="ExternalInput")
    wk_d = nc.dram_tensor("wk", [D, D], bf16, kind="ExternalInput")
    wv_d = nc.dram_tensor("wv", [D, D], bf16, kind="ExternalInput")
    bias_d = nc.dram_tensor("bias", [D], f32, kind="ExternalInput")
    out_d = nc.dram_tensor("out_t", [D, NH], f32, kind="ExternalOutput")

    xrt_r = xrt_d.rearrange("(t p) n -> p t n", p=P)
    xit_r = xit_d.rearrange("(t p) n -> p t n", p=P)
    wqr_r = wqr_d.rearrange("(t p) e -> p t e", p=P)
    wqi_r = wqi_d.rearrange("(t p) e -> p t e", p=P)
    wk_r = wk_d.rearrange("(t p) e -> p t e", p=P)
    wv_r = wv_d.rearrange("(t p) e -> p t e", p=P)
    bias_r = bias_d.rearrange("(t p) -> p t", p=P)
    out_r = out_d.rearrange("(t p) n -> p t n", p=P)

    RG = [[0, 1], [2, 3], [4, 5], [6, 7]]

    with tile.TileContext(nc) as tc:
        with tc.tile_pool(name="outer", bufs=1) as outer, \
             tc.tile_pool(name="dram", bufs=1, space="DRAM") as dram:
            skinv = outer.tile([P, DT], f32, tag="skinv")
            svinv = outer.tile([P, DT], f32, tag="svinv")
            bias_pp = outer.tile([P, DT], f32, tag="bias_pp")
            zero32 = outer.tile([P, 1], f32, tag="zero32")
            ones_bf = outer.tile([P, 1], bf16, tag="ones_bf")
            nc.vector.memset(zero32[:], 0.0)
            nc.vector.memset(ones_bf[:], 1.0)
            nc.sync.dma_start(out=bias_pp[:], in_=bias_r)

            sk2_sb = outer.tile([P, DT], f32, tag="sk2_sb")
            sv2_sb = outer.tile([P, DT], f32, tag="sv2_sb")
            nc.vector.memset(sv2_sb[:], 0.0)

            qt_sb = outer.tile([P, DT, NH], bf16, tag="qt_sb")
            ab_n = outer.tile([P, DT, D], bf16, tag="ab_n")

            cc_in = [dram.tile([P, BDT, AW], bf16, tag=f"cc_in{b}", name=f"cc_in{b}")
                     for b in range(NBLK)]
            cc_out = [dram.tile([P, BDT, AW], bf16, tag=f"cc_out{b}", name=f"cc_out{b}")
                      for b in range(NBLK)]

            ctx_wq = tc.tile_pool(name="wq", bufs=1)
            wq = ctx_wq.__enter__()
            wqr_sb = wq.tile([P, DT, D], bf16, tag="wqr")
            wqi_sb = wq.tile([P, DT, D], bf16, tag="wqi")

            ctx_x = tc.tile_pool(name="xin", bufs=1)
            xp = ctx_x.__enter__()
            xrt_sb = xp.tile([P, DT, NH], bf16, tag="xrt_sb")
            xit_sb = xp.tile([P, DT, NH], bf16, tag="xit_sb")

            # ---------------- Phase A1: V (all n, full e), sv2 ----------------
            with (
                tc.tile_pool(name="wkv", bufs=1) as wkv,
                tc.tile_pool(name="vsb", bufs=1) as vsb,
                tc.tile_pool(name="sqp", bufs=4) as sqp,
                tc.tile_pool(name="ps_v", bufs=3, space="PSUM") as ps_v,
                tc.tile_pool(name="nrm_ps", bufs=2, space="PSUM") as nrm_ps,
            ):
                wk_sb = wkv.tile([P, DT, D], bf16, tag="wk")
                wv_sb = wkv.tile([P, DT, D], bf16, tag="wv")
                v_sb = vsb.tile([P, NTT, D], bf16, tag="v_sb")

                # V-path inputs first (wv + xit), interleaved on both queues;
                # then K-path (wk + xrt), then phase-C weights.
                for t in range(DT):
                    eng = nc.sync if t % 2 else nc.gpsimd
                    eng.dma_start(out=wv_sb[:, t, :], in_=wv_r[:, t, :])
                nc.sync.dma_start(out=xit_sb[:, 0:4, 0:NCK],
                                  in_=xit_r[:, 0:4, 0:NCK])
                nc.gpsimd.dma_start(out=xit_sb[:, 4:8, 0:NCK],
                                    in_=xit_r[:, 4:8, 0:NCK])
                nc.sync.dma_start(out=xit_sb[:, 0:4, NCK:NH],
                                  in_=xit_r[:, 0:4, NCK:NH])
                nc.gpsimd.dma_start(out=xit_sb[:, 4:8, NCK:NH],
                                    in_=xit_r[:, 4:8, NCK:NH])
                for t in range(DT):
                    eng = nc.gpsimd if t % 2 else nc.sync
                    eng.dma_start(out=wk_sb[:, t, :], in_=wk_r[:, t, :])
                nc.sync.dma_start(out=xrt_sb[:, 0:4, :], in_=xrt_r[:, 0:4, :])
                nc.gpsimd.dma_start(out=xrt_sb[:, 4:8, :], in_=xrt_r[:, 4:8, :])
                for t in range(DT):
                    eng = nc.sync if t % 2 else nc.gpsimd
                    eng.dma_start(out=wqr_sb[:, t, :], in_=wqr_r[:, t, :])
                for t in range(DT):
                    eng = nc.gpsimd if t % 2 else nc.sync
                    eng.dma_start(out=wqi_sb[:, t, :], in_=wqi_r[:, t, :])

                for nt in range(NTT):
                    nsl = slice(nt * P, (nt + 1) * P)
                    for eh in range(2):
                        esl = slice(eh * 512, (eh + 1) * 512)
                        vps = ps_v.tile([P, 512], f32, tag="vps")
                        for t in range(DT):
                            nc.tensor.matmul(
                                vps[:], xit_sb[:, t, nsl], wv_sb[:, t, esl],
                                start=(t == 0), stop=(t == DT - 1))
                        nc.vector.tensor_copy(out=v_sb[:, nt, esl], in_=vps[:])
                        sqv = sqp.tile([P, 512], bf16, tag="sq")
                        nc.scalar.activation(out=sqv[:], in_=vps[:],
                                             func=Act.Square,
                                             bias=zero32[:], scale=1.0)
                        npsv = nrm_ps.tile([P, 4], f32, tag="nps")
                        for j in range(4):
                            nc.tensor.matmul(
                                npsv[:, j:j + 1], sqv[:, j * P:(j + 1) * P],
                                ones_bf[:], start=True, stop=True)
                        dsl = slice(eh * 4, (eh + 1) * 4)
                        nc.vector.tensor_add(
                            out=sv2_sb[:, dsl], in0=sv2_sb[:, dsl], in1=npsv[:])

                # ------------ Phase A2: K + kv rows by d-superblock ----------
                with (
                    tc.tile_pool(name="kb", bufs=2) as kbp,
                    tc.tile_pool(name="abp", bufs=2) as abp,
                    tc.tile_pool(name="ps_a", bufs=2, space="PSUM") as ps_a,
                ):
                    for blk in range(NBLK):
                        dsl = slice(blk * BDT * P, (blk + 1) * BDT * P)
                        k_blk = kbp.tile([P, NTT, BDT * P], bf16, tag="k_blk")
                        ab = abp.tile([P, BDT, AW], bf16, tag="ab")
                        for nt in range(NTT):
                            nsl = slice(nt * P, (nt + 1) * P)
                            kps_full = ps_v.tile([P, 512], f32, tag="vps")
                            kps = kps_full[:, 0:BDT * P]
                            for t in range(DT):
                                nc.tensor.matmul(
                                    kps, xrt_sb[:, t, nsl], wk_sb[:, t, dsl],
                                    start=(t == 0), stop=(t == DT - 1))
                            nc.vector.tensor_copy(out=k_blk[:, nt, :], in_=kps)
                            sqk = sqp.tile([P, 512], bf16, tag="sq")
                            nc.scalar.activation(out=sqk[:, 0:BDT * P], in_=kps,
                                                 func=Act.Square,
                                                 bias=zero32[:], scale=1.0)
                            npsk = nrm_ps.tile([P, 4], f32, tag="nps")
                            for j in range(BDT):
                                nc.tensor.matmul(
                                    npsk[:, j:j + 1], sqk[:, j * P:(j + 1) * P],
                                    ones_bf[:], start=True, stop=True)
                            ksl = slice(blk * BDT, blk * BDT + BDT)
                            if nt == 0:
                                nc.vector.tensor_copy(
                                    out=sk2_sb[:, ksl], in_=npsk[:, 0:BDT])
                            else:
                                nc.vector.tensor_add(
                                    out=sk2_sb[:, ksl], in0=sk2_sb[:, ksl],
                                    in1=npsk[:, 0:BDT])

                        for dt2 in range(BDT):
                            for eh in range(2):
                                esl = slice(eh * 512, (eh + 1) * 512)
                                aps = ps_a.tile([P, 512], f32, tag="aps")
                                for nt in range(NTT):
                                    nc.tensor.matmul(
                                        aps[:], k_blk[:, nt, dt2 * P:(dt2 + 1) * P],
                                        v_sb[:, nt, esl], start=(nt == 0),
                                        stop=(nt == NTT - 1))
                                nc.vector.tensor_copy(
                                    out=ab[:, dt2, esl], in_=aps[:])
                        # norms ride in the piece's last two columns
                        ksl = slice(blk * BDT, blk * BDT + BDT)
                        nc.vector.tensor_copy(out=ab[:, :, D], in_=sk2_sb[:, ksl])
                        nc.vector.tensor_copy(out=ab[:, :, D + 1], in_=sv2_sb[:, ksl])
                        nc.gpsimd.dma_start(out=cc_in[blk][:], in_=ab[:])
                        nc.gpsimd.collective_compute(
                            "AllReduce",
                            mybir.AluOpType.add,
                            replica_groups=RG,
                            ins=[cc_in[blk][:].opt()],
                            outs=[cc_out[blk][:].opt()],
                        )

            # ---------------- Phase C1: Q^T (overlaps the CC pipeline) -------
            with (
                tc.tile_pool(name="qrp", bufs=3) as qrp,
                tc.tile_pool(name="ps_q", bufs=4, space="PSUM") as ps_q,
            ):
                for ck in range(CHUNKS):
                    nsl = slice(ck * NCK, (ck + 1) * NCK)
                    for dqt in range(DT):
                        qsl = slice(dqt * P, (dqt + 1) * P)
                        qrps = ps_q.tile([P, NCK], f32, tag="qps")
                        for t in range(DT):
                            nc.tensor.matmul(qrps[:], wqr_sb[:, t, qsl],
                                             xrt_sb[:, t, nsl],
                                             start=(t == 0), stop=(t == DT - 1))
                        qr_sb = qrp.tile([P, NCK], bf16, tag="qr_sb")
                        nc.scalar.activation(out=qr_sb[:], in_=qrps[:],
                                             func=Act.Copy, bias=0.0, scale=1.0)
                        qips = ps_q.tile([P, NCK], f32, tag="qps")
                        for t in range(DT):
                            nc.tensor.matmul(qips[:], wqi_sb[:, t, qsl],
                                             xit_sb[:, t, nsl],
                                             start=(t == 0), stop=(t == DT - 1))
                        nc.vector.tensor_mul(out=qt_sb[:, dqt, nsl],
                                             in0=qips[:], in1=qr_sb[:])

            # ---------------- Phase B: readback pieces, norms, scale ---------
            with tc.tile_pool(name="arp", bufs=2) as arp:
                for blk in range(NBLK):
                    ksl = slice(blk * BDT, blk * BDT + BDT)
                    ar = arp.tile([P, BDT, AW], bf16, tag="ar")
                    nc.sync.dma_start(out=ar[:], in_=cc_out[blk][:])
                    nc.scalar.activation(out=skinv[:, ksl], in_=ar[:, :, D],
                                         func=Act.Sqrt, bias=zero32[:], scale=1.0)
                    nc.vector.tensor_scalar_max(skinv[:, ksl], skinv[:, ksl], EPS)
                    nc.vector.reciprocal(skinv[:, ksl], skinv[:, ksl])
                    nc.scalar.activation(out=svinv[:, ksl], in_=ar[:, :, D + 1],
                                         func=Act.Sqrt, bias=zero32[:], scale=1.0)
                    for dt2 in range(BDT):
                        dt = blk * BDT + dt2
                        nc.vector.tensor_scalar_mul(
                            out=ab_n[:, dt, :], in0=ar[:, dt2, 0:D],
                            scalar1=skinv[:, dt:dt + 1])
                nc.vector.tensor_scalar_max(svinv[:], svinv[:], EPS)
                nc.vector.reciprocal(svinv[:], svinv[:])

            # ---------------- Phase C2: out^T = kv'^T Q^T --------------------
            with (
                tc.tile_pool(name="outp", bufs=4) as outp,
                tc.tile_pool(name="ps_o", bufs=3, space="PSUM") as ps_o,
            ):
                for ck in range(CHUNKS):
                    ns = ck * NCK
                    nsl = slice(ns, ns + NCK)
                    for et in range(DT):
                        esl = slice(et * P, (et + 1) * P)
                        ops_t = ps_o.tile([P, NCK], f32, tag="ops")
                        for dt in range(DT):
                            nc.tensor.matmul(ops_t[:], ab_n[:, dt, esl],
                                             qt_sb[:, dt, nsl],
                                             start=(dt == 0),
                                             stop=(dt == DT - 1))
                        out_sb = outp.tile([P, NCK], f32, tag="out_sb")
                        nc.vector.tensor_scalar(
                            out=out_sb[:], in0=ops_t[:],
                            scalar1=svinv[:, et:et + 1],
                            scalar2=bias_pp[:, et:et + 1],
                            op0=Alu.mult, op1=Alu.add)
                        eng = nc.sync if et % 2 else nc.gpsimd
                        eng.dma_start(out=out_r[:, et, nsl], in_=out_sb[:])

            ctx_x.__exit__(None, None, None)
            ctx_wq.__exit__(None, None, None)

    nc.finalize()
    return nc


def kernel(x_real, x_imag, w_query_real, w_query_imag, w_key, w_value, bias):
    global LAST_EXEC_NS
    import ml_dtypes
    from concourse.bass_utils import run_bass_kernel_spmd

    bf = ml_dtypes.bfloat16
    x_real = np.asarray(x_real, dtype=np.float32)
    x_imag = np.asarray(x_imag, dtype=np.float32)
    wqr = np.asarray(w_query_real, dtype=np.float32).astype(bf)
    wqi = np.asarray(w_query_imag, dtype=np.float32).astype(bf)
    wk = np.asarray(w_key, dtype=np.float32).astype(bf)
    wv = np.asarray(w_value, dtype=np.float32).astype(bf)
    bias = np.ascontiguousarray(np.asarray(bias, dtype=np.float32))

    nc = _CACHE.get("nc")
    if nc is None:
        nc = _build_bass()
        _CACHE["nc"] = nc

    in_maps = []
    for c in range(8):
        b, h = c // 2, c % 2
        sl = slice(h * NH, (h + 1) * NH)
        in_maps.append({
            "xrt": x_real[b, sl].T.astype(bf, order="C"),
            "xit": x_imag[b, sl].T.astype(bf, order="C"),
            "wqr": wqr, "wqi": wqi, "wk": wk, "wv": wv,
            "bias": bias,
        })

    res = run_bass_kernel_spmd(nc, in_maps, list(range(8)))
    LAST_EXEC_NS = res.exec_time_ns

    out = np.empty((B, N, D), dtype=np.float32)
    for c in range(8):
        b, h = c // 2, c % 2
        out[b, h * NH:(h + 1) * NH, :] = np.asarray(res.results[c]["out_t"]).T
    return out


# revision 3
# speedup vs baseline: 1.0515x; 1.0124x over previous
"""Trainium2 Bass kernel for nn_BilinearFeedForward — n-split + pipelined CC.

Sharding: 8 cores = (batch b) x (n-half h).  Each core does the ideal
6.45G MACs: K, V, Q, out for its own 1024 rows; partial kv^T and column
norms pairwise AllReduce'd in bf16.

Phase A is restructured V-first, then K and the kv rows by d-superblock
(4 blocks x 2 d-tiles), so each block's partial-kv rows + norms are
final early and its 525KB AllReduce piece is issued mid-phase-A.  The
4-piece CC pipeline overlaps phase A's tail and the whole Q phase.

All matmuls bf16 (same PE rate as f32r, half DMA/SBUF), PSUM fp32.
"""

import os
import sys
import numpy as np

for _p in ("/opt/trn_rl_repo", "/root/.axon_site/_ro/trn_rl_repo"):
    if _p not in sys.path and os.path.isdir(_p):
        sys.path.append(_p)

try:
    import antenv.axon_hooks  # noqa: F401
except Exception:
    import types

    try:
        import antenv

        _hooks = types.ModuleType("antenv.axon_hooks")
        _hooks._hook = None
        _hooks.get_axon_ntff_profile_hook = lambda: _hooks._hook

        def _set_hook(h):
            _hooks._hook = h

        _hooks.set_axon_ntff_profile_hook = _set_hook
        sys.modules["antenv.axon_hooks"] = _hooks
        antenv.axon_hooks = _hooks
    except Exception:
        pass

B, N, D = 4, 2048, 1024
NH = N // 2       # rows per core
P = 128
DT = D // P       # 8 d-tiles
NTT = NH // P     # 8 n-tiles total
NCK = 512         # n-chunk for V / Q / out phases
CHUNKS = NH // NCK  # 2
NBLK = 4          # kv d-superblocks
BDT = DT // NBLK  # 2 d-tiles per block
AW = D + 2        # CC row width: 1024 kv cols + sk2 + sv2
EPS = 1e-5

_CACHE = {}
LAST_EXEC_NS = None


def _build_bass():
    import concourse.bacc as bacc
    import concourse.tile as tile
    import concourse.mybir as mybir

    f32 = mybir.dt.float32
    bf16 = mybir.dt.bfloat16
    Act = mybir.ActivationFunctionType
    Alu = mybir.AluOpType

    nc = bacc.Bacc(num_devices=8)

    xrt_d = nc.dram_tensor("xrt", [D, NH], bf16, kind="ExternalInput")
    xit_d = nc.dram_tensor("xit", [D, NH], bf16, kind="ExternalInput")
    wqr_d = nc.dram_tensor("wqr", [D, D], bf16, kind="ExternalInput")
    wqi_d = nc.dram_tensor("wqi", [D, D], bf16, kind="ExternalInput")
    wk_d = nc.dram_tensor("wk", [D, D], bf16, kind="ExternalInput")
    wv_d = nc.dram_tensor("wv", [D, D], bf16, kind="ExternalInput")
    bias_d = nc.dram_tensor("bias", [D], f32, kind="ExternalInput")
    out_d = nc.dram_tensor("out_t", [D, NH], f32, kind="ExternalOutput")

    xrt_r = xrt_d.rearrange("(t p) n -> p t n", p=P)
    xit_r = xit_d.rearrange("(t p) n -> p t n", p=P)
    wqr_r = wqr_d.rearrange("(t p) e -> p t e", p=P)
    wqi_r = wqi_d.rearrange("(t p) e -> p t e", p=P)
    wk_r = wk_d.rearrange("(t p) e -> p t e", p=P)
    wv_r = wv_d.rearrange("(t p) e -> p t e", p=P)
    bias_r = bias_d.rearrange("(t p) -> p t", p=P)
    out_r = out_d.rearrange("(t p) n -> p t n", p=P)

    RG = [[0, 1], [2, 3], [4, 5], [6, 7]]

    with tile.TileContext(nc) as tc:
        with tc.tile_pool(name="outer", bufs=1) as outer, \
             tc.tile_pool(name="dram", bufs=1, space="DRAM") as dram:
            skinv = outer.tile([P, DT], f32, tag="skinv")
            svinv = outer.tile([P, DT], f32, tag="svinv")
            bias_pp = outer.tile([P, DT], f32, tag="bias_pp")
            zero32 = outer.tile([P, 1], f32, tag="zero32")
            ones_bf = outer.tile([P, 1], bf16, tag="ones_bf")
            nc.vector.memset(zero32[:], 0.0)
            nc.vector.memset(ones_bf[:], 1.0)
            nc.sync.dma_start(out=bias_pp[:], in_=bias_r)

            sk2_sb = outer.tile([P, DT], f32, tag="sk2_sb")
            sv2_sb = outer.tile([P, DT], f32, tag="sv2_sb")
            nc.vector.memset(sv2_sb[:], 0.0)

            qt_sb = outer.tile([P, DT, NH], bf16, tag="qt_sb")
            ab_n = outer.tile([P, DT, D], bf16, tag="ab_n")

            cc_in = [dram.tile([P, BDT, AW], bf16, tag=f"cc_in{b}", name=f"cc_in{b}")
                     for b in range(NBLK)]
            cc_out = [dram.tile([P, BDT, AW], bf16, tag=f"cc_out{b}", name=f"cc_out{b}")
                      for b in range(NBLK)]

            ctx_wq = tc.tile_pool(name="wq", bufs=1)
            wq = ctx_wq.__enter__()
            wqr_sb = wq.tile([P, DT, D], bf16, tag="wqr")
            wqi_sb = wq.tile([P, DT, D], bf16, tag="wqi")

            ctx_x = tc.tile_pool(name="xin", bufs=1)
            xp = ctx_x.__enter__()
            xrt_sb = xp.tile([P, DT, NH], bf16, tag="xrt_sb")
            xit_sb = xp.tile([P, DT, NH], bf16, tag="xit_sb")

            # ---------------- Phase A1: V (all n, full e), sv2 ----------------
            with (
                tc.tile_pool(name="wkv", bufs=1) as wkv,
                tc.tile_pool(name="vsb", bufs=1) as vsb,
                tc.tile_pool(name="sqp", bufs=4) as sqp,
                tc.tile_pool(name="ps_v", bufs=3, space="PSUM") as ps_v,
                tc.tile_pool(name="nrm_ps", bufs=2, space="PSUM") as nrm_ps,
            ):
                wk_sb = wkv.tile([P, DT, D], bf16, tag="wk")
                wv_sb = wkv.tile([P, DT, D], bf16, tag="wv")
                v_sb = vsb.tile([P, NTT, D], bf16, tag="v_sb")

                # load order = first-use order: wv e-half0, xit, wv e-half1,
                # wk, xrt, then phase-C weights; per-t granularity so the
                # accumulation chains start as tiles land.
                for t in range(DT):
                    eng = nc.sync if t % 2 else nc.gpsimd
                    eng.dma_start(out=wv_sb[:, t, 0:512], in_=wv_r[:, t, 0:512])
                for t in range(DT):
                    eng = nc.gpsimd if t % 2 else nc.sync
                    eng.dma_start(out=xit_sb[:, t, :], in_=xit_r[:, t, :])
                for t in range(DT):
                    eng = nc.sync if t % 2 else nc.gpsimd
                    eng.dma_start(out=wv_sb[:, t, 512:D], in_=wv_r[:, t, 512:D])
                for t in range(DT):
                    eng = nc.gpsimd if t % 2 else nc.sync
                    eng.dma_start(out=wk_sb[:, t, :], in_=wk_r[:, t, :])
                for t in range(DT):
                    eng = nc.sync if t % 2 else nc.gpsimd
                    eng.dma_start(out=xrt_sb[:, t, :], in_=xrt_r[:, t, :])
                for t in range(DT):
                    eng = nc.gpsimd if t % 2 else nc.sync
                    eng.dma_start(out=wqr_sb[:, t, :], in_=wqr_r[:, t, :])
                for t in range(DT):
                    eng = nc.sync if t % 2 else nc.gpsimd
                    eng.dma_start(out=wqi_sb[:, t, :], in_=wqi_r[:, t, :])

                for eh in range(2):
                    esl = slice(eh * 512, (eh + 1) * 512)
                    for nt in range(NTT):
                        nsl = slice(nt * P, (nt + 1) * P)
                        vps = ps_v.tile([P, 512], f32, tag="vps")
                        for t in range(DT):
                            nc.tensor.matmul(
                                vps[:], xit_sb[:, t, nsl], wv_sb[:, t, esl],
                                start=(t == 0), stop=(t == DT - 1))
                        nc.vector.tensor_copy(out=v_sb[:, nt, esl], in_=vps[:])
                        sqv = sqp.tile([P, 512], bf16, tag="sq")
                        nc.scalar.activation(out=sqv[:], in_=vps[:],
                                             func=Act.Square,
                                             bias=zero32[:], scale=1.0)
                        npsv = nrm_ps.tile([P, 4], f32, tag="nps")
                        for j in range(4):
                            nc.tensor.matmul(
                                npsv[:, j:j + 1], sqv[:, j * P:(j + 1) * P],
                                ones_bf[:], start=True, stop=True)
                        dsl = slice(eh * 4, (eh + 1) * 4)
                        nc.vector.tensor_add(
                            out=sv2_sb[:, dsl], in0=sv2_sb[:, dsl], in1=npsv[:])

                # ------------ Phase A2: K + kv rows by d-superblock ----------
                with (
                    tc.tile_pool(name="kb", bufs=2) as kbp,
                    tc.tile_pool(name="abp", bufs=2) as abp,
                    tc.tile_pool(name="ps_a", bufs=2, space="PSUM") as ps_a,
                ):
                    for blk in range(NBLK):
                        dsl = slice(blk * BDT * P, (blk + 1) * BDT * P)
                        k_blk = kbp.tile([P, NTT, BDT * P], bf16, tag="k_blk")
                        ab = abp.tile([P, BDT, AW], bf16, tag="ab")
                        for nt in range(NTT):
                            nsl = slice(nt * P, (nt + 1) * P)
                            kps_full = ps_v.tile([P, 512], f32, tag="vps")
                            kps = kps_full[:, 0:BDT * P]
                            for t in range(DT):
                                nc.tensor.matmul(
                                    kps, xrt_sb[:, t, nsl], wk_sb[:, t, dsl],
                                    start=(t == 0), stop=(t == DT - 1))
                            nc.vector.tensor_copy(out=k_blk[:, nt, :], in_=kps)
                            sqk = sqp.tile([P, 512], bf16, tag="sq")
                            nc.scalar.activation(out=sqk[:, 0:BDT * P], in_=kps,
                                                 func=Act.Square,
                                                 bias=zero32[:], scale=1.0)
                            npsk = nrm_ps.tile([P, 4], f32, tag="nps")
                            for j in range(BDT):
                                nc.tensor.matmul(
                                    npsk[:, j:j + 1], sqk[:, j * P:(j + 1) * P],
                                    ones_bf[:], start=True, stop=True)
                            ksl = slice(blk * BDT, blk * BDT + BDT)
                            if nt == 0:
                                nc.vector.tensor_copy(
                                    out=sk2_sb[:, ksl], in_=npsk[:, 0:BDT])
                            else:
                                nc.vector.tensor_add(
                                    out=sk2_sb[:, ksl], in0=sk2_sb[:, ksl],
                                    in1=npsk[:, 0:BDT])

                        for dt2 in range(BDT):
                            for eh in range(2):
                                esl = slice(eh * 512, (eh + 1) * 512)
                                aps = ps_a.tile([P, 512], f32, tag="aps")
                                for nt in range(NTT):
                                    nc.tensor.matmul(
                                        aps[:], k_blk[:, nt, dt2 * P:(dt2 + 1) * P],
                                        v_sb[:, nt, esl], start=(nt == 0),
                                        stop=(nt == NTT - 1))
                                nc.vector.tensor_copy(
                                    out=ab[:, dt2, esl], in_=aps[:])
                        # norms ride in the piece's last two columns
                        ksl = slice(blk * BDT, blk * BDT + BDT)
                        nc.vector.tensor_copy(out=ab[:, :, D], in_=sk2_sb[:, ksl])
                        nc.vector.tensor_copy(out=ab[:, :, D + 1], in_=sv2_sb[:, ksl])
                        nc.scalar.dma_start(out=cc_in[blk][:], in_=ab[:])
                        nc.gpsimd.collective_compute(
                            "AllReduce",
                            mybir.AluOpType.add,
                            replica_groups=RG,
                            ins=[cc_in[blk][:].opt()],
                            outs=[cc_out[blk][:].opt()],
                        )

            # ---------------- Phase C1: Q^T (overlaps the CC pipeline) -------
            with (
                tc.tile_pool(name="qrp", bufs=3) as qrp,
                tc.tile_pool(name="ps_q", bufs=4, space="PSUM") as ps_q,
            ):
                for ck in range(CHUNKS):
                    nsl = slice(ck * NCK, (ck + 1) * NCK)
                    for dqt in range(DT):
                        qsl = slice(dqt * P, (dqt + 1) * P)
                        qrps = ps_q.tile([P, NCK], f32, tag="qps")
                        for t in range(DT):
                            nc.tensor.matmul(qrps[:], wqr_sb[:, t, qsl],
                                             xrt_sb[:, t, nsl],
                                             start=(t == 0), stop=(t == DT - 1))
                        qr_sb = qrp.tile([P, NCK], bf16, tag="qr_sb")
                        nc.scalar.activation(out=qr_sb[:], in_=qrps[:],
                                             func=Act.Copy, bias=0.0, scale=1.0)
                        qips = ps_q.tile([P, NCK], f32, tag="qps")
                        for t in range(DT):
                            nc.tensor.matmul(qips[:], wqi_sb[:, t, qsl],
                                             xit_sb[:, t, nsl],
                                             start=(t == 0), stop=(t == DT - 1))
                        nc.vector.tensor_mul(out=qt_sb[:, dqt, nsl],
                                             in0=qips[:], in1=qr_sb[:])

            # ---------------- Phase B: readback pieces, norms, scale ---------
            with tc.tile_pool(name="arp", bufs=2) as arp:
                for blk in range(NBLK):
                    ksl = slice(blk * BDT, blk * BDT + BDT)
                    ar = arp.tile([P, BDT, AW], bf16, tag="ar")
                    nc.gpsimd.dma_start(out=ar[:], in_=cc_out[blk][:])
                    nc.scalar.activation(out=skinv[:, ksl], in_=ar[:, :, D],
                                         func=Act.Sqrt, bias=zero32[:], scale=1.0)
                    nc.vector.tensor_scalar_max(skinv[:, ksl], skinv[:, ksl], EPS)
                    nc.vector.reciprocal(skinv[:, ksl], skinv[:, ksl])
                    nc.scalar.activation(out=svinv[:, ksl], in_=ar[:, :, D + 1],
                                         func=Act.Sqrt, bias=zero32[:], scale=1.0)
                    for dt2 in range(BDT):
                        dt = blk * BDT + dt2
                        nc.vector.tensor_scalar_mul(
                            out=ab_n[:, dt, :], in0=ar[:, dt2, 0:D],
                            scalar1=skinv[:, dt:dt + 1])
                nc.vector.tensor_scalar_max(svinv[:], svinv[:], EPS)
                nc.vector.reciprocal(svinv[:], svinv[:])

            # ---------------- Phase C2: out^T = kv'^T Q^T --------------------
            with (
                tc.tile_pool(name="outp", bufs=4) as outp,
                tc.tile_pool(name="ps_o", bufs=3, space="PSUM") as ps_o,
            ):
                for ck in range(CHUNKS):
                    ns = ck * NCK
                    nsl = slice(ns, ns + NCK)
                    for et in range(DT):
                        esl = slice(et * P, (et + 1) * P)
                        ops_t = ps_o.tile([P, NCK], f32, tag="ops")
                        for dt in range(DT):
                            nc.tensor.matmul(ops_t[:], ab_n[:, dt, esl],
                                             qt_sb[:, dt, nsl],
                                             start=(dt == 0),
                                             stop=(dt == DT - 1))
                        out_sb = outp.tile([P, NCK], f32, tag="out_sb")
                        nc.vector.tensor_scalar(
                            out=out_sb[:], in0=ops_t[:],
                            scalar1=svinv[:, et:et + 1],
                            scalar2=bias_pp[:, et:et + 1],
                            op0=Alu.mult, op1=Alu.add)
                        eng = nc.sync if et % 2 else nc.gpsimd
                        eng.dma_start(out=out_r[:, et, nsl], in_=out_sb[:])

            ctx_x.__exit__(None, None, None)
            ctx_wq.__exit__(None, None, None)

    nc.finalize()
    return nc


def kernel(x_real, x_imag, w_query_real, w_query_imag, w_key, w_value, bias):
    global LAST_EXEC_NS
    import ml_dtypes
    from concourse.bass_utils import run_bass_kernel_spmd

    bf = ml_dtypes.bfloat16
    x_real = np.asarray(x_real, dtype=np.float32)
    x_imag = np.asarray(x_imag, dtype=np.float32)
    wqr = np.asarray(w_query_real, dtype=np.float32).astype(bf)
    wqi = np.asarray(w_query_imag, dtype=np.float32).astype(bf)
    wk = np.asarray(w_key, dtype=np.float32).astype(bf)
    wv = np.asarray(w_value, dtype=np.float32).astype(bf)
    bias = np.ascontiguousarray(np.asarray(bias, dtype=np.float32))

    nc = _CACHE.get("nc")
    if nc is None:
        nc = _build_bass()
        _CACHE["nc"] = nc

    in_maps = []
    for c in range(8):
        b, h = c // 2, c % 2
        sl = slice(h * NH, (h + 1) * NH)
        in_maps.append({
            "xrt": x_real[b, sl].T.astype(bf, order="C"),
            "xit": x_imag[b, sl].T.astype(bf, order="C"),
            "wqr": wqr, "wqi": wqi, "wk": wk, "wv": wv,
            "bias": bias,
        })

    res = run_bass_kernel_spmd(nc, in_maps, list(range(8)))
    LAST_EXEC_NS = res.exec_time_ns

    out = np.empty((B, N, D), dtype=np.float32)
    for c in range(8):
        b, h = c // 2, c % 2
        out[b, h * NH:(h + 1) * NH, :] = np.asarray(res.results[c]["out_t"]).T
    return out


# revision 4
# speedup vs baseline: 1.1150x; 1.0604x over previous
"""Trainium2 Bass kernel for nn_BilinearFeedForward — n-split + pipelined CC.

Sharding: 8 cores = (batch b) x (n-half h).  Each core does the ideal
6.45G MACs: K, V, Q, out for its own 1024 rows; partial kv^T and column
norms pairwise AllReduce'd in bf16.

Phase order interleaves the collective pipeline with independent PE
work:  A1 (V, all n) -> C1a (Q chunk 0) -> A2 (K + kv rows by
d-superblock, each block's 525KB AllReduce piece issued as it
finalizes) -> C1b (Q chunk 1, covers the last CC piece + readback)
-> C2 (out).  Readback tiles live in their own early-allocated pool so
the piece readbacks don't inherit false SBUF-reuse dependencies.

All matmuls bf16 (same PE rate as f32r, half DMA/SBUF), PSUM fp32.
"""

import os
import sys
import numpy as np

for _p in ("/opt/trn_rl_repo", "/root/.axon_site/_ro/trn_rl_repo"):
    if _p not in sys.path and os.path.isdir(_p):
        sys.path.append(_p)

try:
    import antenv.axon_hooks  # noqa: F401
except Exception:
    import types

    try:
        import antenv

        _hooks = types.ModuleType("antenv.axon_hooks")
        _hooks._hook = None
        _hooks.get_axon_ntff_profile_hook = lambda: _hooks._hook

        def _set_hook(h):
            _hooks._hook = h

        _hooks.set_axon_ntff_profile_hook = _set_hook
        sys.modules["antenv.axon_hooks"] = _hooks
        antenv.axon_hooks = _hooks
    except Exception:
        pass

B, N, D = 4, 2048, 1024
NH = N // 2       # rows per core
P = 128
DT = D // P       # 8 d-tiles
NTT = NH // P     # 8 n-tiles total
NCK = 512         # n-chunk for Q / out phases
CHUNKS = NH // NCK  # 2
BLKS = [3, 3, 1, 1]   # kv d-superblock sizes (d-tiles); big first, small last
NBLK = len(BLKS)
BOFF = [sum(BLKS[:i]) for i in range(NBLK)]
AW = D + 2        # CC row width: 1024 kv cols + sk2 + sv2
EPS = 1e-5

_CACHE = {}
LAST_EXEC_NS = None


def _build_bass():
    import concourse.bacc as bacc
    import concourse.tile as tile
    import concourse.mybir as mybir

    f32 = mybir.dt.float32
    bf16 = mybir.dt.bfloat16
    Act = mybir.ActivationFunctionType
    Alu = mybir.AluOpType

    nc = bacc.Bacc(num_devices=8)

    xrt_d = nc.dram_tensor("xrt", [D, NH], bf16, kind="ExternalInput")
    xit_d = nc.dram_tensor("xit", [D, NH], bf16, kind="ExternalInput")
    wqr_d = nc.dram_tensor("wqr", [D, D], bf16, kind="ExternalInput")
    wqi_d = nc.dram_tensor("wqi", [D, D], bf16, kind="ExternalInput")
    wk_d = nc.dram_tensor("wk", [D, D], bf16, kind="ExternalInput")
    wv_d = nc.dram_tensor("wv", [D, D], bf16, kind="ExternalInput")
    bias_d = nc.dram_tensor("bias", [D], f32, kind="ExternalInput")
    out_d = nc.dram_tensor("out_t", [D, NH], f32, kind="ExternalOutput")

    xrt_r = xrt_d.rearrange("(t p) n -> p t n", p=P)
    xit_r = xit_d.rearrange("(t p) n -> p t n", p=P)
    wqr_r = wqr_d.rearrange("(t p) e -> p t e", p=P)
    wqi_r = wqi_d.rearrange("(t p) e -> p t e", p=P)
    wk_r = wk_d.rearrange("(t p) e -> p t e", p=P)
    wv_r = wv_d.rearrange("(t p) e -> p t e", p=P)
    bias_r = bias_d.rearrange("(t p) -> p t", p=P)
    out_r = out_d.rearrange("(t p) n -> p t n", p=P)

    RG = [[0, 1], [2, 3], [4, 5], [6, 7]]

    with tile.TileContext(nc) as tc:
        with tc.tile_pool(name="outer", bufs=1) as outer, \
             tc.tile_pool(name="dram", bufs=1, space="DRAM") as dram, \
             tc.tile_pool(name="arp", bufs=1) as arp:
            skinv = outer.tile([P, DT], f32, tag="skinv")
            svinv = outer.tile([P, DT], f32, tag="svinv")
            bias_pp = outer.tile([P, DT], f32, tag="bias_pp")
            zero32 = outer.tile([P, 1], f32, tag="zero32")
            ones_bf = outer.tile([P, 1], bf16, tag="ones_bf")
            nc.vector.memset(zero32[:], 0.0)
            nc.vector.memset(ones_bf[:], 1.0)
            nc.sync.dma_start(out=bias_pp[:], in_=bias_r)

            sk2_sb = outer.tile([P, DT], f32, tag="sk2_sb")
            sv2_sb = outer.tile([P, DT], f32, tag="sv2_sb")
            nc.vector.memset(sv2_sb[:], 0.0)

            qt_sb = outer.tile([P, DT, NH], bf16, tag="qt_sb")
            ab_n = outer.tile([P, DT, D], bf16, tag="ab_n")
            # readback tiles: one per piece, own pool so no reuse deps
            ar = [arp.tile([P, BLKS[b], AW], bf16, tag=f"ar{b}", name=f"ar{b}")
                  for b in range(NBLK)]

            cc_in = [dram.tile([P, BLKS[b], AW], bf16, tag=f"cc_in{b}", name=f"cc_in{b}")
                     for b in range(NBLK)]
            cc_out = [dram.tile([P, BLKS[b], AW], bf16, tag=f"cc_out{b}", name=f"cc_out{b}")
                      for b in range(NBLK)]

            ctx_wq = tc.tile_pool(name="wq", bufs=1)
            wq = ctx_wq.__enter__()
            wqr_sb = wq.tile([P, DT, D], bf16, tag="wqr")
            wqi_sb = wq.tile([P, DT, D], bf16, tag="wqi")

            ctx_x = tc.tile_pool(name="xin", bufs=1)
            xp = ctx_x.__enter__()
            xrt_sb = xp.tile([P, DT, NH], bf16, tag="xrt_sb")
            xit_sb = xp.tile([P, DT, NH], bf16, tag="xit_sb")

            ctx_v = tc.tile_pool(name="vsb", bufs=1)
            vp = ctx_v.__enter__()
            v_sb = vp.tile([P, NTT, D], bf16, tag="v_sb")
            wk_sb = vp.tile([P, DT, D], bf16, tag="wk")
            wv_sb = vp.tile([P, DT, D], bf16, tag="wv")

            # load order = first-use order: wv e-half0, xit, wv e-half1,
            # xrt, wqr, wqi, wk; per-t granularity so accumulation chains
            # start as tiles land.
            for t in range(DT):
                eng = nc.sync if t % 2 else nc.gpsimd
                eng.dma_start(out=wv_sb[:, t, 0:512], in_=wv_r[:, t, 0:512])
            for t in range(DT):
                eng = nc.gpsimd if t % 2 else nc.sync
                eng.dma_start(out=xit_sb[:, t, :], in_=xit_r[:, t, :])
            for t in range(DT):
                eng = nc.sync if t % 2 else nc.gpsimd
                eng.dma_start(out=wv_sb[:, t, 512:D], in_=wv_r[:, t, 512:D])
            for t in range(DT):
                eng = nc.gpsimd if t % 2 else nc.sync
                eng.dma_start(out=wk_sb[:, t, :], in_=wk_r[:, t, :])
            for t in range(DT):
                eng = nc.sync if t % 2 else nc.gpsimd
                eng.dma_start(out=xrt_sb[:, t, :], in_=xrt_r[:, t, :])
            for t in range(DT):
                eng = nc.gpsimd if t % 2 else nc.sync
                eng.dma_start(out=wqr_sb[:, t, :], in_=wqr_r[:, t, :])
            for t in range(DT):
                eng = nc.sync if t % 2 else nc.gpsimd
                eng.dma_start(out=wqi_sb[:, t, :], in_=wqi_r[:, t, :])

            with tc.tile_pool(name="sqp", bufs=4) as sqp, \
                 tc.tile_pool(name="nrm_ps", bufs=2, space="PSUM") as nrm_ps:

                # -------- Phase A1: V (all n, full e), sv2 -------------------
                with tc.tile_pool(name="ps_v", bufs=3, space="PSUM") as ps_v:
                    for eh in range(2):
                        esl = slice(eh * 512, (eh + 1) * 512)
                        for nt in range(NTT):
                            nsl = slice(nt * P, (nt + 1) * P)
                            vps = ps_v.tile([P, 512], f32, tag="vps")
                            for t in range(DT):
                                nc.tensor.matmul(
                                    vps[:], xit_sb[:, t, nsl], wv_sb[:, t, esl],
                                    start=(t == 0), stop=(t == DT - 1))
                            nc.vector.tensor_copy(out=v_sb[:, nt, esl], in_=vps[:])
                            sqv = sqp.tile([P, 512], bf16, tag="sq")
                            nc.scalar.activation(out=sqv[:], in_=vps[:],
                                                 func=Act.Square,
                                                 bias=zero32[:], scale=1.0)
                            npsv = nrm_ps.tile([P, 4], f32, tag="nps")
                            for j in range(4):
                                nc.tensor.matmul(
                                    npsv[:, j:j + 1], sqv[:, j * P:(j + 1) * P],
                                    ones_bf[:], start=True, stop=True)
                            dsl = slice(eh * 4, (eh + 1) * 4)
                            nc.vector.tensor_add(
                                out=sv2_sb[:, dsl], in0=sv2_sb[:, dsl], in1=npsv[:])

                # -------- Phase A2: K + kv rows by d-superblock + CC ---------
                with (
                    tc.tile_pool(name="kb", bufs=2) as kbp,
                    tc.tile_pool(name="abp", bufs=2) as abp,
                    tc.tile_pool(name="ps_k", bufs=3, space="PSUM") as ps_k,
                    tc.tile_pool(name="ps_a", bufs=2, space="PSUM") as ps_a,
                ):
                    for blk in range(NBLK):
                        bdt = BLKS[blk]
                        dsl = slice(BOFF[blk] * P, (BOFF[blk] + bdt) * P)
                        ksl = slice(BOFF[blk], BOFF[blk] + bdt)
                        k_blk = kbp.tile([P, NTT, bdt * P], bf16,
                                         tag=f"k_blk{bdt}")
                        ab = abp.tile([P, bdt, AW], bf16, tag=f"ab{bdt}")
                        for nt in range(NTT):
                            nsl = slice(nt * P, (nt + 1) * P)
                            kps_full = ps_k.tile([P, 512], f32, tag="kps")
                            kps = kps_full[:, 0:bdt * P]
                            for t in range(DT):
                                nc.tensor.matmul(
                                    kps, xrt_sb[:, t, nsl], wk_sb[:, t, dsl],
                                    start=(t == 0), stop=(t == DT - 1))
                            nc.vector.tensor_copy(out=k_blk[:, nt, :], in_=kps)
                            sqk = sqp.tile([P, 512], bf16, tag="sq")
                            nc.scalar.activation(out=sqk[:, 0:bdt * P], in_=kps,
                                                 func=Act.Square,
                                                 bias=zero32[:], scale=1.0)
                            npsk = nrm_ps.tile([P, 4], f32, tag="nps")
                            for j in range(bdt):
                                nc.tensor.matmul(
                                    npsk[:, j:j + 1], sqk[:, j * P:(j + 1) * P],
                                    ones_bf[:], start=True, stop=True)
                            if nt == 0:
                                nc.vector.tensor_copy(
                                    out=sk2_sb[:, ksl], in_=npsk[:, 0:bdt])
                            else:
                                nc.vector.tensor_add(
                                    out=sk2_sb[:, ksl], in0=sk2_sb[:, ksl],
                                    in1=npsk[:, 0:bdt])

                        for dt2 in range(bdt):
                            for eh in range(2):
                                esl = slice(eh * 512, (eh + 1) * 512)
                                aps = ps_a.tile([P, 512], f32, tag="aps")
                                for nt in range(NTT):
                                    nc.tensor.matmul(
                                        aps[:], k_blk[:, nt, dt2 * P:(dt2 + 1) * P],
                                        v_sb[:, nt, esl], start=(nt == 0),
                                        stop=(nt == NTT - 1))
                                nc.vector.tensor_copy(
                                    out=ab[:, dt2, esl], in_=aps[:])
                        # norms ride in the piece's last two columns
                        nc.vector.tensor_copy(out=ab[:, :, D], in_=sk2_sb[:, ksl])
                        nc.vector.tensor_copy(out=ab[:, :, D + 1], in_=sv2_sb[:, ksl])
                        nc.scalar.dma_start(out=cc_in[blk][:], in_=ab[:])
                        nc.gpsimd.collective_compute(
                            "AllReduce",
                            mybir.AluOpType.add,
                            replica_groups=RG,
                            ins=[cc_in[blk][:].opt()],
                            outs=[cc_out[blk][:].opt()],
                        )
                        # pipelined readback + norm finalize + row scale
                        nc.sync.dma_start(out=ar[blk][:], in_=cc_out[blk][:])
                        nc.scalar.activation(out=skinv[:, ksl], in_=ar[blk][:, :, D],
                                             func=Act.Sqrt, bias=zero32[:],
                                             scale=1.0)
                        nc.vector.tensor_scalar_max(skinv[:, ksl], skinv[:, ksl],
                                                    EPS)
                        nc.vector.reciprocal(skinv[:, ksl], skinv[:, ksl])
                        nc.scalar.activation(out=svinv[:, ksl],
                                             in_=ar[blk][:, :, D + 1],
                                             func=Act.Sqrt, bias=zero32[:],
                                             scale=1.0)
                        for dt2 in range(bdt):
                            dt = BOFF[blk] + dt2
                            nc.vector.tensor_scalar_mul(
                                out=ab_n[:, dt, :], in0=ar[blk][:, dt2, 0:D],
                                scalar1=skinv[:, dt:dt + 1])
                    nc.vector.tensor_scalar_max(svinv[:], svinv[:], EPS)
                    nc.vector.reciprocal(svinv[:], svinv[:])

            # -------- Phase C1: Q^T (covers the CC pipeline tail) ------------
            with tc.tile_pool(name="qrb", bufs=3) as qrb, \
                 tc.tile_pool(name="ps_qb", bufs=4, space="PSUM") as ps_qb:
              for ck in range(CHUNKS):
                nsl = slice(ck * NCK, (ck + 1) * NCK)
                for dqt in range(DT):
                    qsl = slice(dqt * P, (dqt + 1) * P)
                    qrps = ps_qb.tile([P, NCK], f32, tag="qps")
                    for t in range(DT):
                        nc.tensor.matmul(qrps[:], wqr_sb[:, t, qsl],
                                         xrt_sb[:, t, nsl],
                                         start=(t == 0), stop=(t == DT - 1))
                    qr_sb = qrb.tile([P, NCK], bf16, tag="qr_sb")
                    nc.scalar.activation(out=qr_sb[:], in_=qrps[:],
                                         func=Act.Copy, bias=0.0, scale=1.0)
                    qips = ps_qb.tile([P, NCK], f32, tag="qps")
                    for t in range(DT):
                        nc.tensor.matmul(qips[:], wqi_sb[:, t, qsl],
                                         xit_sb[:, t, nsl],
                                         start=(t == 0), stop=(t == DT - 1))
                    nc.vector.tensor_mul(out=qt_sb[:, dqt, nsl],
                                         in0=qips[:], in1=qr_sb[:])

            # -------- Phase C2: out^T = kv'^T Q^T ----------------------------
            with (
                tc.tile_pool(name="outp", bufs=4) as outp,
                tc.tile_pool(name="ps_o", bufs=3, space="PSUM") as ps_o,
            ):
                for ck in range(CHUNKS):
                    ns = ck * NCK
                    nsl = slice(ns, ns + NCK)
                    for et in range(DT):
                        esl = slice(et * P, (et + 1) * P)
                        ops_t = ps_o.tile([P, NCK], f32, tag="ops")
                        for dt in range(DT):
                            nc.tensor.matmul(ops_t[:], ab_n[:, dt, esl],
                                             qt_sb[:, dt, nsl],
                                             start=(dt == 0),
                                             stop=(dt == DT - 1))
                        out_sb = outp.tile([P, NCK], f32, tag="out_sb")
                        nc.vector.tensor_scalar(
                            out=out_sb[:], in0=ops_t[:],
                            scalar1=svinv[:, et:et + 1],
                            scalar2=bias_pp[:, et:et + 1],
                            op0=Alu.mult, op1=Alu.add)
                        eng = nc.gpsimd if et % 2 else nc.sync
                        eng.dma_start(out=out_r[:, et, nsl], in_=out_sb[:])

            ctx_v.__exit__(None, None, None)
            ctx_x.__exit__(None, None, None)
            ctx_wq.__exit__(None, None, None)

    nc.finalize()
    return nc


def kernel(x_real, x_imag, w_query_real, w_query_imag, w_key, w_value, bias):
    global LAST_EXEC_NS
    import ml_dtypes
    from concourse.bass_utils import run_bass_kernel_spmd

    bf = ml_dtypes.bfloat16
    x_real = np.asarray(x_real, dtype=np.float32)
    x_imag = np.asarray(x_imag, dtype=np.float32)
    wqr = np.asarray(w_query_real, dtype=np.float32).astype(bf)
    wqi = np.asarray(w_query_imag, dtype=np.float32).astype(bf)
    wk = np.asarray(w_key, dtype=np.float32).astype(bf)
    wv = np.asarray(w_value, dtype=np.float32).astype(bf)
    bias = np.ascontiguousarray(np.asarray(bias, dtype=np.float32))

    nc = _CACHE.get("nc")
    if nc is None:
        nc = _build_bass()
        _CACHE["nc"] = nc

    in_maps = []
    for c in range(8):
        b, h = c // 2, c % 2
        sl = slice(h * NH, (h + 1) * NH)
        in_maps.append({
            "xrt": x_real[b, sl].T.astype(bf, order="C"),
            "xit": x_imag[b, sl].T.astype(bf, order="C"),
            "wqr": wqr, "wqi": wqi, "wk": wk, "wv": wv,
            "bias": bias,
        })

    res = run_bass_kernel_spmd(nc, in_maps, list(range(8)))
    LAST_EXEC_NS = res.exec_time_ns

    out = np.empty((B, N, D), dtype=np.float32)
    for c in range(8):
        b, h = c // 2, c % 2
        out[b, h * NH:(h + 1) * NH, :] = np.asarray(res.results[c]["out_t"]).T
    return out


# revision 5
# speedup vs baseline: 1.1198x; 1.0043x over previous
"""Trainium2 Bass kernel for nn_BilinearFeedForward — n-split + pipelined CC.

Sharding: 8 cores = (batch b) x (n-half h).  Each core does the ideal
6.45G MACs: K, V, Q, out for its own 1024 rows; partial kv^T and column
norms pairwise AllReduce'd in bf16.

Phase order interleaves the collective pipeline with independent PE
work:  A1 (V, all n) -> C1a (Q chunk 0) -> A2 (K + kv rows by
d-superblock, each block's 525KB AllReduce piece issued as it
finalizes) -> C1b (Q chunk 1, covers the last CC piece + readback)
-> C2 (out).  Readback tiles live in their own early-allocated pool so
the piece readbacks don't inherit false SBUF-reuse dependencies.

All matmuls bf16 (same PE rate as f32r, half DMA/SBUF), PSUM fp32.
"""

import os
import sys
import numpy as np

for _p in ("/opt/trn_rl_repo", "/root/.axon_site/_ro/trn_rl_repo"):
    if _p not in sys.path and os.path.isdir(_p):
        sys.path.append(_p)

try:
    import antenv.axon_hooks  # noqa: F401
except Exception:
    import types

    try:
        import antenv

        _hooks = types.ModuleType("antenv.axon_hooks")
        _hooks._hook = None
        _hooks.get_axon_ntff_profile_hook = lambda: _hooks._hook

        def _set_hook(h):
            _hooks._hook = h

        _hooks.set_axon_ntff_profile_hook = _set_hook
        sys.modules["antenv.axon_hooks"] = _hooks
        antenv.axon_hooks = _hooks
    except Exception:
        pass

B, N, D = 4, 2048, 1024
NH = N // 2       # rows per core
P = 128
DT = D // P       # 8 d-tiles
NTT = NH // P     # 8 n-tiles total
NCK = 512         # n-chunk for Q / out phases
CHUNKS = NH // NCK  # 2
BLKS = [1, 2, 2, 3]   # kv d-superblock sizes (d-tiles); small first so the CC pipeline starts early
NBLK = len(BLKS)
BOFF = [sum(BLKS[:i]) for i in range(NBLK)]
AW = D + 2        # CC row width: 1024 kv cols + sk2 + sv2
EPS = 1e-5

_CACHE = {}
LAST_EXEC_NS = None


def _build_bass():
    import concourse.bacc as bacc
    import concourse.tile as tile
    import concourse.mybir as mybir

    f32 = mybir.dt.float32
    bf16 = mybir.dt.bfloat16
    Act = mybir.ActivationFunctionType
    Alu = mybir.AluOpType

    nc = bacc.Bacc(num_devices=8)

    xrt_d = nc.dram_tensor("xrt", [D, NH], bf16, kind="ExternalInput")
    xit_d = nc.dram_tensor("xit", [D, NH], bf16, kind="ExternalInput")
    wqr_d = nc.dram_tensor("wqr", [D, D], bf16, kind="ExternalInput")
    wqi_d = nc.dram_tensor("wqi", [D, D], bf16, kind="ExternalInput")
    wk_d = nc.dram_tensor("wk", [D, D], bf16, kind="ExternalInput")
    wv_d = nc.dram_tensor("wv", [D, D], bf16, kind="ExternalInput")
    bias_d = nc.dram_tensor("bias", [D], f32, kind="ExternalInput")
    out_d = nc.dram_tensor("out_t", [D, NH], f32, kind="ExternalOutput")

    xrt_r = xrt_d.rearrange("(t p) n -> p t n", p=P)
    xit_r = xit_d.rearrange("(t p) n -> p t n", p=P)
    wqr_r = wqr_d.rearrange("(t p) e -> p t e", p=P)
    wqi_r = wqi_d.rearrange("(t p) e -> p t e", p=P)
    wk_r = wk_d.rearrange("(t p) e -> p t e", p=P)
    wv_r = wv_d.rearrange("(t p) e -> p t e", p=P)
    bias_r = bias_d.rearrange("(t p) -> p t", p=P)
    out_r = out_d.rearrange("(t p) n -> p t n", p=P)

    RG = [[0, 1], [2, 3], [4, 5], [6, 7]]

    with tile.TileContext(nc) as tc:
        with tc.tile_pool(name="outer", bufs=1) as outer, \
             tc.tile_pool(name="dram", bufs=1, space="DRAM") as dram, \
             tc.tile_pool(name="arp", bufs=1) as arp:
            skinv = outer.tile([P, DT], f32, tag="skinv")
            svinv = outer.tile([P, DT], f32, tag="svinv")
            bias_pp = outer.tile([P, DT], f32, tag="bias_pp")
            zero32 = outer.tile([P, 1], f32, tag="zero32")
            ones_bf = outer.tile([P, 1], bf16, tag="ones_bf")
            nc.vector.memset(zero32[:], 0.0)
            nc.vector.memset(ones_bf[:], 1.0)
            nc.scalar.dma_start(out=bias_pp[:], in_=bias_r)

            sk2_sb = outer.tile([P, DT], f32, tag="sk2_sb")
            sv2_sb = outer.tile([P, DT], f32, tag="sv2_sb")
            nc.vector.memset(sv2_sb[:], 0.0)

            qt_sb = outer.tile([P, DT, NH], bf16, tag="qt_sb")
            ab_n = outer.tile([P, DT, D], bf16, tag="ab_n")
            # readback tiles: one per piece, own pool so no reuse deps
            ar = [arp.tile([P, BLKS[b], AW], bf16, tag=f"ar{b}", name=f"ar{b}")
                  for b in range(NBLK)]

            cc_in = [dram.tile([P, BLKS[b], AW], bf16, tag=f"cc_in{b}", name=f"cc_in{b}")
                     for b in range(NBLK)]
            cc_out = [dram.tile([P, BLKS[b], AW], bf16, tag=f"cc_out{b}", name=f"cc_out{b}")
                      for b in range(NBLK)]

            ctx_wq = tc.tile_pool(name="wq", bufs=1)
            wq = ctx_wq.__enter__()
            wqr_sb = wq.tile([P, DT, D], bf16, tag="wqr")
            wqi_sb = wq.tile([P, DT, D], bf16, tag="wqi")

            ctx_x = tc.tile_pool(name="xin", bufs=1)
            xp = ctx_x.__enter__()
            xrt_sb = xp.tile([P, DT, NH], bf16, tag="xrt_sb")
            xit_sb = xp.tile([P, DT, NH], bf16, tag="xit_sb")

            ctx_v = tc.tile_pool(name="vsb", bufs=1)
            vp = ctx_v.__enter__()
            v_sb = vp.tile([P, NTT, D], bf16, tag="v_sb")
            wk_sb = vp.tile([P, DT, D], bf16, tag="wk")
            wv_sb = vp.tile([P, DT, D], bf16, tag="wv")

            # load order = first-use order: wv e-half0, xit, wv e-half1,
            # xrt, wqr, wqi, wk; per-t granularity so accumulation chains
            # start as tiles land.
            for t in range(DT):
                eng = nc.sync if t % 2 else nc.gpsimd
                eng.dma_start(out=wv_sb[:, t, 0:512], in_=wv_r[:, t, 0:512])
            for t in range(DT):
                eng = nc.gpsimd if t % 2 else nc.sync
                eng.dma_start(out=xit_sb[:, t, 0:NCK], in_=xit_r[:, t, 0:NCK])
            for t in range(DT):
                eng = nc.sync if t % 2 else nc.gpsimd
                eng.dma_start(out=xit_sb[:, t, NCK:NH], in_=xit_r[:, t, NCK:NH])
            for t in range(DT):
                eng = nc.gpsimd if t % 2 else nc.sync
                eng.dma_start(out=wv_sb[:, t, 512:D], in_=wv_r[:, t, 512:D])
            for t in range(DT):
                eng = nc.gpsimd if t % 2 else nc.sync
                eng.dma_start(out=wk_sb[:, t, :], in_=wk_r[:, t, :])
            for t in range(DT):
                eng = nc.sync if t % 2 else nc.gpsimd
                eng.dma_start(out=xrt_sb[:, t, :], in_=xrt_r[:, t, :])
            for t in range(DT):
                eng = nc.gpsimd if t % 2 else nc.sync
                eng.dma_start(out=wqr_sb[:, t, :], in_=wqr_r[:, t, :])
            for t in range(DT):
                eng = nc.sync if t % 2 else nc.gpsimd
                eng.dma_start(out=wqi_sb[:, t, :], in_=wqi_r[:, t, :])

            with tc.tile_pool(name="sqp", bufs=4) as sqp, \
                 tc.tile_pool(name="nrm_ps", bufs=2, space="PSUM") as nrm_ps:

                # -------- Phase A1: V (all n, full e), sv2 -------------------
                with tc.tile_pool(name="ps_v", bufs=3, space="PSUM") as ps_v:
                    for eh in range(2):
                        esl = slice(eh * 512, (eh + 1) * 512)
                        for nt in range(NTT):
                            nsl = slice(nt * P, (nt + 1) * P)
                            vps = ps_v.tile([P, 512], f32, tag="vps")
                            for t in range(DT):
                                nc.tensor.matmul(
                                    vps[:], xit_sb[:, t, nsl], wv_sb[:, t, esl],
                                    start=(t == 0), stop=(t == DT - 1))
                            nc.vector.tensor_copy(out=v_sb[:, nt, esl], in_=vps[:])
                            sqv = sqp.tile([P, 512], bf16, tag="sq")
                            nc.scalar.activation(out=sqv[:], in_=vps[:],
                                                 func=Act.Square,
                                                 bias=zero32[:], scale=1.0)
                            npsv = nrm_ps.tile([P, 4], f32, tag="nps")
                            for j in range(4):
                                nc.tensor.matmul(
                                    npsv[:, j:j + 1], sqv[:, j * P:(j + 1) * P],
                                    ones_bf[:], start=True, stop=True)
                            dsl = slice(eh * 4, (eh + 1) * 4)
                            nc.vector.tensor_add(
                                out=sv2_sb[:, dsl], in0=sv2_sb[:, dsl], in1=npsv[:])

                # -------- Phase A2: K + kv rows by d-superblock + CC ---------
                with (
                    tc.tile_pool(name="kb", bufs=2) as kbp,
                    tc.tile_pool(name="abp", bufs=2) as abp,
                    tc.tile_pool(name="ps_k", bufs=3, space="PSUM") as ps_k,
                    tc.tile_pool(name="ps_a", bufs=2, space="PSUM") as ps_a,
                ):
                    for blk in range(NBLK):
                        bdt = BLKS[blk]
                        dsl = slice(BOFF[blk] * P, (BOFF[blk] + bdt) * P)
                        ksl = slice(BOFF[blk], BOFF[blk] + bdt)
                        k_full = kbp.tile([P, NTT, max(BLKS) * P], bf16,
                                          tag="k_blk")
                        ab_full = abp.tile([P, max(BLKS), AW], bf16, tag="ab")
                        ab = ab_full[:, 0:bdt, :]
                        for nt in range(NTT):
                            nsl = slice(nt * P, (nt + 1) * P)
                            kps_full = ps_k.tile([P, 512], f32, tag="kps")
                            kps = kps_full[:, 0:bdt * P]
                            for t in range(DT):
                                nc.tensor.matmul(
                                    kps, xrt_sb[:, t, nsl], wk_sb[:, t, dsl],
                                    start=(t == 0), stop=(t == DT - 1))
                            nc.vector.tensor_copy(out=k_full[:, nt, 0:bdt * P], in_=kps)
                            sqk = sqp.tile([P, 512], bf16, tag="sq")
                            nc.scalar.activation(out=sqk[:, 0:bdt * P], in_=kps,
                                                 func=Act.Square,
                                                 bias=zero32[:], scale=1.0)
                            npsk = nrm_ps.tile([P, 4], f32, tag="nps")
                            for j in range(bdt):
                                nc.tensor.matmul(
                                    npsk[:, j:j + 1], sqk[:, j * P:(j + 1) * P],
                                    ones_bf[:], start=True, stop=True)
                            if nt == 0:
                                nc.vector.tensor_copy(
                                    out=sk2_sb[:, ksl], in_=npsk[:, 0:bdt])
                            else:
                                nc.vector.tensor_add(
                                    out=sk2_sb[:, ksl], in0=sk2_sb[:, ksl],
                                    in1=npsk[:, 0:bdt])

                        for dt2 in range(bdt):
                            for eh in range(2):
                                esl = slice(eh * 512, (eh + 1) * 512)
                                aps = ps_a.tile([P, 512], f32, tag="aps")
                                for nt in range(NTT):
                                    nc.tensor.matmul(
                                        aps[:], k_full[:, nt, dt2 * P:(dt2 + 1) * P],
                                        v_sb[:, nt, esl], start=(nt == 0),
                                        stop=(nt == NTT - 1))
                                nc.vector.tensor_copy(
                                    out=ab[:, dt2, esl], in_=aps[:])
                        # norms ride in the piece's last two columns
                        nc.vector.tensor_copy(out=ab[:, :, D], in_=sk2_sb[:, ksl])
                        nc.vector.tensor_copy(out=ab[:, :, D + 1], in_=sv2_sb[:, ksl])
                        nc.scalar.dma_start(out=cc_in[blk][:], in_=ab[:])
                        nc.gpsimd.collective_compute(
                            "AllReduce",
                            mybir.AluOpType.add,
                            replica_groups=RG,
                            ins=[cc_in[blk][:].opt()],
                            outs=[cc_out[blk][:].opt()],
                        )
                        # pipelined readback + norm finalize + row scale
                        nc.sync.dma_start(out=ar[blk][:], in_=cc_out[blk][:])
                        nc.scalar.activation(out=skinv[:, ksl], in_=ar[blk][:, :, D],
                                             func=Act.Sqrt, bias=zero32[:],
                                             scale=1.0)
                        nc.vector.tensor_scalar_max(skinv[:, ksl], skinv[:, ksl],
                                                    EPS)
                        nc.vector.reciprocal(skinv[:, ksl], skinv[:, ksl])
                        nc.scalar.activation(out=svinv[:, ksl],
                                             in_=ar[blk][:, :, D + 1],
                                             func=Act.Sqrt, bias=zero32[:],
                                             scale=1.0)
                        for dt2 in range(bdt):
                            dt = BOFF[blk] + dt2
                            nc.vector.tensor_scalar_mul(
                                out=ab_n[:, dt, :], in0=ar[blk][:, dt2, 0:D],
                                scalar1=skinv[:, dt:dt + 1])
                    nc.vector.tensor_scalar_max(svinv[:], svinv[:], EPS)
                    nc.vector.reciprocal(svinv[:], svinv[:])

            # -------- Phase C1: Q^T (covers the CC pipeline tail) ------------
            with tc.tile_pool(name="qrb", bufs=3) as qrb, \
                 tc.tile_pool(name="ps_qb", bufs=4, space="PSUM") as ps_qb:
              for ck in range(CHUNKS):
                nsl = slice(ck * NCK, (ck + 1) * NCK)
                for dqt in range(DT):
                    qsl = slice(dqt * P, (dqt + 1) * P)
                    qrps = ps_qb.tile([P, NCK], f32, tag="qps")
                    for t in range(DT):
                        nc.tensor.matmul(qrps[:], wqr_sb[:, t, qsl],
                                         xrt_sb[:, t, nsl],
                                         start=(t == 0), stop=(t == DT - 1))
                    qr_sb = qrb.tile([P, NCK], bf16, tag="qr_sb")
                    nc.scalar.activation(out=qr_sb[:], in_=qrps[:],
                                         func=Act.Copy, bias=0.0, scale=1.0)
                    qips = ps_qb.tile([P, NCK], f32, tag="qps")
                    for t in range(DT):
                        nc.tensor.matmul(qips[:], wqi_sb[:, t, qsl],
                                         xit_sb[:, t, nsl],
                                         start=(t == 0), stop=(t == DT - 1))
                    nc.vector.tensor_mul(out=qt_sb[:, dqt, nsl],
                                         in0=qips[:], in1=qr_sb[:])

            # -------- Phase C2: out^T = kv'^T Q^T ----------------------------
            with (
                tc.tile_pool(name="outp", bufs=4) as outp,
                tc.tile_pool(name="ps_o", bufs=3, space="PSUM") as ps_o,
            ):
                for ck in range(CHUNKS):
                    ns = ck * NCK
                    nsl = slice(ns, ns + NCK)
                    for et in range(DT):
                        esl = slice(et * P, (et + 1) * P)
                        ops_t = ps_o.tile([P, NCK], f32, tag="ops")
                        for dt in range(DT):
                            nc.tensor.matmul(ops_t[:], ab_n[:, dt, esl],
                                             qt_sb[:, dt, nsl],
                                             start=(dt == 0),
                                             stop=(dt == DT - 1))
                        out_sb = outp.tile([P, NCK], f32, tag="out_sb")
                        nc.vector.tensor_scalar(
                            out=out_sb[:], in0=ops_t[:],
                            scalar1=svinv[:, et:et + 1],
                            scalar2=bias_pp[:, et:et + 1],
                            op0=Alu.mult, op1=Alu.add)
                        eng = nc.gpsimd if et % 2 else nc.sync
                        eng.dma_start(out=out_r[:, et, nsl], in_=out_sb[:])

            ctx_v.__exit__(None, None, None)
            ctx_x.__exit__(None, None, None)
            ctx_wq.__exit__(None, None, None)

    nc.finalize()
    return nc


def kernel(x_real, x_imag, w_query_real, w_query_imag, w_key, w_value, bias):
    global LAST_EXEC_NS
    import ml_dtypes
    from concourse.bass_utils import run_bass_kernel_spmd

    bf = ml_dtypes.bfloat16
    x_real = np.asarray(x_real, dtype=np.float32)
    x_imag = np.asarray(x_imag, dtype=np.float32)
    wqr = np.asarray(w_query_real, dtype=np.float32).astype(bf)
    wqi = np.asarray(w_query_imag, dtype=np.float32).astype(bf)
    wk = np.asarray(w_key, dtype=np.float32).astype(bf)
    wv = np.asarray(w_value, dtype=np.float32).astype(bf)
    bias = np.ascontiguousarray(np.asarray(bias, dtype=np.float32))

    nc = _CACHE.get("nc")
    if nc is None:
        nc = _build_bass()
        _CACHE["nc"] = nc

    in_maps = []
    for c in range(8):
        b, h = c // 2, c % 2
        sl = slice(h * NH, (h + 1) * NH)
        in_maps.append({
            "xrt": x_real[b, sl].T.astype(bf, order="C"),
            "xit": x_imag[b, sl].T.astype(bf, order="C"),
            "wqr": wqr, "wqi": wqi, "wk": wk, "wv": wv,
            "bias": bias,
        })

    res = run_bass_kernel_spmd(nc, in_maps, list(range(8)))
    LAST_EXEC_NS = res.exec_time_ns

    out = np.empty((B, N, D), dtype=np.float32)
    for c in range(8):
        b, h = c // 2, c % 2
        out[b, h * NH:(h + 1) * NH, :] = np.asarray(res.results[c]["out_t"]).T
    return out
